# revision 53
# baseline (speedup 1.0000x reference)
"""Distributed TRN2 Bass kernel for a dynamic-int8-quantized transformer encoder.

v2: column-parallel sharding, 2 heads x both batches per core.
  - core c: heads {2c, 2c+1} (256 qkv out dims), io token slice = flat block c
    (batch c//4, tokens (c%4)*512..+512).
  - x quantized per-core on its 512-token slice, AllGathered all-8 (RDH) as
    integer-valued bf16; Wq/Wk/Wv column slices stay LOCAL (no weight
    AllGather); Wp quantized sharded + AllGathered (overlapped mid-kernel).
  - projections column-parallel: q/k/v for ALL 4096 tokens x this core's
    2 heads; attention fully local per (batch, head) - 4 pairs per core.
  - after attention: AllToAll (all-8) redistributes quantized out to
    token-parallel; final projection token-parallel vs gathered Wp.
  - int8 fake-quant matmuls computed exactly as integer-valued bf16 matmuls
    (fp32 PSUM), dequant scales on PSUM eviction; global per-tensor amaxes via
    tiny AllReduce-max collectives.
  - softmax without max-subtraction; pass2 recomputes QK^T transposed with an
    extra k=3 ones-matmul adding c'_i = -ln(S_i)/s_qk (bf16 hi+mid+lo split),
    so round(exp(s_qk*(a_int + c'_i) - ln s_attn)) yields int8 probs directly.
  - round-to-nearest-even: x + 1.5*2^23 - 1.5*2^23 in fp32 (matches jnp.round).
"""

import math
import os
import sys
from contextlib import ExitStack

import numpy as np

sys.path.insert(0, "/opt/trn_rl_repo")

B = 2
S = 2048
H = 2048
NH = 16
D = 128
NC = 8
GROUP = 4          # io-slice cores per batch
HPC = 2            # heads per core
OS = 256           # per-core qkv out-dim slice (2 heads)
NT = 4096          # total tokens (B*S)
TB = 2048          # tokens per batch
TLOC = 512         # io token slice per core
OSL = 256          # per-core Wp out-dim slice
Q_MUL = 1.0 / math.sqrt(D)
RMAGIC = 12582912.0  # 1.5 * 2**23

_COMPILED = {}


def _build(debug=False):
    import concourse.mybir as mybir
    import concourse.tile as tile
    from concourse import bacc
    from concourse import bass_isa

    f32 = mybir.dt.float32
    bf16 = mybir.dt.bfloat16
    AF = mybir.ActivationFunctionType
    OP = mybir.AluOpType
    AX = mybir.AxisListType

    nc = bacc.Bacc(None, target_bir_lowering=False, debug=False, num_devices=NC)

    x_T = nc.declare_dram_parameter("x_T", [H, TLOC], f32, isOutput=False)
    wqkv = nc.declare_dram_parameter("wqkv", [3, H, OS], f32, isOutput=False)
    wp_sl = nc.declare_dram_parameter("wp_sl", [H, OSL], f32, isOutput=False)
    bqk = nc.declare_dram_parameter("bqk", [2, 128, HPC], f32, isOutput=False)
    bv_b = nc.declare_dram_parameter("bv_b", [128, OS], f32, isOutput=False)
    bp_t = nc.declare_dram_parameter("bp_t", [128, 16], f32, isOutput=False)
    ident = nc.declare_dram_parameter("ident", [128, 128], f32, isOutput=False)
    out_ext = nc.declare_dram_parameter("out", [H, TLOC], f32, isOutput=True)
    dbg = {}
    if debug:
        for name, shape in [
            ("d_scales", [1, 16]),
            ("d_S", [128, 64]), ("d_M", [128, 64]),
            ("d_qdeq", [128, HPC, NT]), ("d_kdeq", [128, HPC, NT]),
            ("d_vdeq", [32, 128, OS]), ("d_outT", [128, HPC, NT]),
        ]:
            dbg[name] = nc.declare_dram_parameter(name, shape, f32, isOutput=True)
        for name, shape in [
            ("d_xt", [128, 16, TLOC]), ("d_wq", [128, 16, OS]),
        ]:
            dbg[name] = nc.declare_dram_parameter(name, shape, bf16, isOutput=True)

    allg = [list(range(NC))]

    with tile.TileContext(nc) as tc, ExitStack() as top:
        dram = top.enter_context(tc.tile_pool(name="dram", bufs=1, space="DRAM"))
        # x AllGather bounce (single 2MB chunk), all-8 gather
        xq_b = dram.tile([16, 128, TLOC], bf16, name="xqb")
        xg = dram.tile([NC, 16, 128, TLOC], bf16, addr_space="Shared", name="xg")
        # v_deq spill (f32, streamed back for quantization after AR2)
        vdq = dram.tile([32, 128, OS], f32, name="vdq")
        # wp bounce + gather (stationary-tiled 4KB read rows)
        wpb = dram.tile([2, 128, 16, 128], bf16, name="wpb")
        wpg = dram.tile([NC, 2, 128, 16, 128], bf16, addr_space="Shared", name="wpg")
        # A2A for out_int redistribution (all-8)
        a2a_i = dram.tile([NC, HPC, 128, TLOC], bf16, name="a2ai")
        a2a_o = dram.tile([NC, HPC, 128, TLOC], bf16, name="a2ao")
        # c' rows (f32) for pass2 column bias: [bh-pair][i]
        rt_f32 = dram.tile([4, TB], f32, name="rtf32")
        # AllReduce bounces
        ar1a_i = dram.tile([1, 3], f32)
        ar1a_o = dram.tile([1, 3], f32, addr_space="Shared")
        ar1b_i = dram.tile([1, 2], f32)
        ar1b_o = dram.tile([1, 2], f32, addr_space="Shared")
        ar2_i = dram.tile([1, 3], f32)
        ar2_o = dram.tile([1, 3], f32, addr_space="Shared")
        ar3_i = dram.tile([1, 1], f32)
        ar3_o = dram.tile([1, 1], f32, addr_space="Shared")
        ar4_i = dram.tile([1, 1], f32)
        ar4_o = dram.tile([1, 1], f32, addr_space="Shared")

        const = top.enter_context(tc.tile_pool(name="const", bufs=1))
        sc = top.enter_context(tc.tile_pool(name="scal", bufs=1))
        sbuf = top.enter_context(tc.tile_pool(name="sbuf_main", bufs=1))
        qpool = top.enter_context(tc.tile_pool(name="qscratch", bufs=3))

        ones3 = const.tile([3, 128], bf16)
        nc.vector.memset(ones3[:], 1.0)

        scal = sc.tile([128, 96], f32, name="scal")
        _col = [0]

        def cols(n):
            c0 = _col[0]
            _col[0] += n
            assert _col[0] <= 96
            return scal[:, c0 : c0 + n]

        parts = sc.tile([128, 192], f32, name="parts")
        _pcol = [0]

        def pcols(n):
            c0 = _pcol[0]
            _pcol[0] += n
            assert _pcol[0] <= 192
            return parts[:, c0 : c0 + n]

        def p_reduce_max(part_col):
            red = cols(1)
            nc.gpsimd.partition_all_reduce(
                red, part_col, channels=128, reduce_op=bass_isa.ReduceOp.max
            )
            return red

        def bcast(src1n):
            b = cols(src1n.shape[-1])
            nc.gpsimd.partition_broadcast(b, src1n)
            return b

        def rnd(out_ap, in_ap):
            nc.vector.tensor_scalar(
                out_ap, in_ap, RMAGIC, RMAGIC, op0=OP.add, op1=OP.subtract
            )

        # persistent tiles (stack order: freed in reverse)
        q_int, q_int_free = tc.tile([128, HPC, NT], bf16, name="q_int")
        k_int, k_int_free = tc.tile([128, HPC, NT], bf16, name="k_int")
        wq_int, wq_int_free = tc.tile([128, 16, OS], bf16, name="wq_int")
        wk_int, wk_int_free = tc.tile([128, 16, OS], bf16, name="wk_int")
        wv_int, wv_int_free = tc.tile([128, 16, OS], bf16, name="wv_int")

        # ============ S0: load x + W slices resident; amaxes; AR1 ============
        w_f32, w_f32_free = tc.tile([128, 3, 16 * OS], f32, name="w_f32")
        wp_f32, wp_f32_free = tc.tile([128, 16 * OSL], f32, name="wp_f32")
        x_f32, x_f32_free = tc.tile([128, 16, TLOC], f32, name="x_f32")

        nc.sync.dma_start(out=x_f32[:], in_=x_T.rearrange("(ht p) t -> p ht t", p=128))
        nc.sync.dma_start(
            out=w_f32[:].rearrange("p w (ht o) -> p w ht o", o=OS),
            in_=wqkv.rearrange("w (ht p) o -> p w ht o", p=128),
        )
        nc.sync.dma_start(
            out=wp_f32[:].rearrange("p (ht o) -> p ht o", o=OSL),
            in_=wp_sl.rearrange("(ht p) o -> p ht o", p=128),
        )

        xa = pcols(1)
        nc.vector.tensor_reduce(
            xa, x_f32[:].rearrange("p a b -> p (a b)"), AX.X, OP.max,
            apply_absolute_value=True,
        )
        xag = p_reduce_max(xa)
        nc.sync.dma_start(out=ar1a_i[:, 0:1], in_=xag[0:1, :])
        for w, dst_ap in ((0, ar1a_i[:, 1:2]), (1, ar1a_i[:, 2:3])):
            wa = pcols(1)
            nc.vector.tensor_reduce(
                wa, w_f32[:, w, :], AX.X, OP.max, apply_absolute_value=True
            )
            wag = p_reduce_max(wa)
            nc.sync.dma_start(out=dst_ap, in_=wag[0:1, :])
        nc.gpsimd.collective_compute(
            "AllReduce", OP.max, replica_groups=allg,
            ins=[ar1a_i[:].opt()], outs=[ar1a_o[:].opt()],
        )
        # wv / wp amaxes (AR1b emitted AFTER the x AllGather on the cc queue)
        wa = pcols(1)
        nc.vector.tensor_reduce(
            wa, w_f32[:, 2, :], AX.X, OP.max, apply_absolute_value=True
        )
        wag = p_reduce_max(wa)
        nc.sync.dma_start(out=ar1b_i[:, 0:1], in_=wag[0:1, :])
        wa2 = pcols(1)
        nc.vector.tensor_reduce(
            wa2, wp_f32[:], AX.X, OP.max, apply_absolute_value=True
        )
        wag2 = p_reduce_max(wa2)
        nc.sync.dma_start(out=ar1b_i[:, 1:2], in_=wag2[0:1, :])

        # scales: slots [x, wq, wk, wv, wp] — x/wq/wk part right after AR1a
        g5 = cols(5)[0:1, :]
        s5 = cols(5)[0:1, :]
        i5 = cols(5)[0:1, :]
        i5b = cols(5)
        sxw = cols(3)[0:1, :]   # s_x*s_wq, s_x*s_wk, s_x*s_wv
        sxwb = cols(3)

        nc.sync.dma_start(out=g5[:, 0:3], in_=ar1a_o[:])
        nc.vector.tensor_scalar(
            s5[:, 0:3], g5[:, 0:3], 1.0 / 127.0, 1e-8, op0=OP.mult, op1=OP.max
        )
        nc.vector.reciprocal(i5[:, 0:3], s5[:, 0:3])
        nc.gpsimd.partition_broadcast(i5b[:, 0:3], i5[:, 0:3])
        nc.vector.tensor_mul(sxw[:, 0:1], s5[:, 0:1], s5[:, 1:2])
        nc.vector.tensor_mul(sxw[:, 1:2], s5[:, 0:1], s5[:, 2:3])
        nc.gpsimd.partition_broadcast(sxwb[:, 0:2], sxw[:, 0:2])

        # ============ S1: quantize x slice -> bounce -> AGx ============
        for ch in range(4):
            xsl = x_f32[:, ch * 4 : (ch + 1) * 4, :].rearrange("p a b -> p (a b)")
            xm = qpool.tile([128, TB], f32, name="xm", tag="qs_f32")
            nc.scalar.activation(xm[:], xsl, AF.Copy, scale=i5b[:, 0:1])
            xi = qpool.tile([128, TB], bf16, name="xi", tag="qs_bf16")
            rnd(xi[:], xm[:])
            for hh in range(4):
                nc.sync.dma_start(
                    out=xq_b[ch * 4 + hh],
                    in_=xi[:, hh * TLOC : (hh + 1) * TLOC],
                )
        nc.gpsimd.collective_compute(
            "AllGather", OP.bypass, replica_groups=allg,
            ins=[xq_b[:].opt()], outs=[xg[:].opt()],
        )
        x_f32_free()

        # AR1b on the cc queue after AGx's trigger; wire time overlaps AGx/quant
        nc.gpsimd.collective_compute(
            "AllReduce", OP.max, replica_groups=allg,
            ins=[ar1b_i[:].opt()], outs=[ar1b_o[:].opt()],
        )
        nc.sync.dma_start(out=g5[:, 3:5], in_=ar1b_o[:])
        nc.vector.tensor_scalar(
            s5[:, 3:5], g5[:, 3:5], 1.0 / 127.0, 1e-8, op0=OP.mult, op1=OP.max
        )
        nc.vector.reciprocal(i5[:, 3:5], s5[:, 3:5])
        nc.gpsimd.partition_broadcast(i5b[:, 3:5], i5[:, 3:5])
        nc.vector.tensor_mul(sxw[:, 2:3], s5[:, 0:1], s5[:, 3:4])
        nc.gpsimd.partition_broadcast(sxwb[:, 2:3], sxw[:, 2:3])

        # ============ S2: quantize weights (big chunks, from SBUF) ============
        for w, dst, scol in ((0, wq_int, 1), (1, wk_int, 2)):
            for ch in range(2):
                csl = slice(ch * 8 * OS, (ch + 1) * 8 * OS)
                wm = qpool.tile([128, TB], f32, name="wm", tag="qs_f32")
                nc.scalar.activation(
                    wm[:], w_f32[:, w, csl], AF.Copy, scale=i5b[:, scol : scol + 1]
                )
                rnd(
                    dst[:, ch * 8 : (ch + 1) * 8, :].rearrange("p a b -> p (a b)"),
                    wm[:],
                )
        for ch in range(2):
            csl = slice(ch * 8 * OS, (ch + 1) * 8 * OS)
            wm = qpool.tile([128, TB], f32, name="wmv", tag="qs_f32")
            nc.scalar.activation(wm[:], w_f32[:, 2, csl], AF.Copy, scale=i5b[:, 3:4])
            rnd(
                wv_int[:, ch * 8 : (ch + 1) * 8, :].rearrange("p a b -> p (a b)"),
                wm[:],
            )
        # wp slice -> DRAM bounce (AllGather queued after AR2)
        wpb_r = wpb.rearrange("half p ht o -> p half ht o")
        for ch in range(2):
            csl = slice(ch * 8 * OSL, (ch + 1) * 8 * OSL)
            wm = qpool.tile([128, TB], f32, name="wmp", tag="qs_f32")
            nc.scalar.activation(wm[:], wp_f32[:, csl], AF.Copy, scale=i5b[:, 4:5])
            wi = qpool.tile([128, TB], bf16, name="wip", tag="qs_bf16")
            rnd(wi[:], wm[:])
            for hh in range(8):
                nc.sync.dma_start(
                    out=wpb_r[:, :, ch * 8 + hh, :],
                    in_=wi[:, hh * OSL : (hh + 1) * OSL].rearrange(
                        "p (half o) -> p half o", o=128
                    ),
                )
        wp_f32_free()
        w_f32_free()

        # ============ S4: projections (v, q, k) column-parallel ============
        q_deq, q_deq_free = tc.tile([128, HPC, NT], f32, name="q_deq")
        k_deq, k_deq_free = tc.tile([128, HPC, NT], f32, name="k_deq")
        bq_sb = const.tile([128, HPC], f32)
        nc.sync.dma_start(out=bq_sb[:], in_=bqk[0])
        bk_sb = const.tile([128, HPC], f32)
        nc.sync.dma_start(out=bk_sb[:], in_=bqk[1])
        bv_sb = const.tile([128, OS], f32)
        nc.sync.dma_start(out=bv_sb[:], in_=bv_b[:, :])
        bp_sb = const.tile([128, 16], f32)
        nc.sync.dma_start(out=bp_sb[:], in_=bp_t[:, :])
        ident_sb = const.tile([128, 128], f32)
        nc.sync.dma_start(out=ident_sb[:], in_=ident[:, :])

        qa_parts = pcols(16)
        ka_parts = pcols(16)
        va_parts = pcols(32)

        with tc.tile_pool(name="xt", bufs=2) as xt_pool, \
             tc.tile_pool(name="v_psum", bufs=2, space="PSUM") as v_psum, \
             tc.tile_pool(name="qk_psum", bufs=6, space="PSUM") as qk_psum:
            for tt in range(NC):
                xt = xt_pool.tile([128, 16, TLOC], bf16, name="xt")
                nc.sync.dma_start(
                    out=xt[:], in_=xg[tt].rearrange("h p t -> p h t")
                )
                if debug and tt == 0:
                    nc.sync.dma_start(out=dbg["d_xt"][:], in_=xt[:])
                    nc.sync.dma_start(out=dbg["d_wq"][:], in_=wq_int[:])

                def xtile(ht):
                    return xt[:, ht, :]

                # v first: out [tok, o] per 128-token block
                for tc4 in range(4):
                    ps = v_psum.tile([128, OS], f32, name="ps_v")
                    for ht in range(16):
                        nc.tensor.matmul(
                            ps[:], xtile(ht)[:, tc4 * 128 : (tc4 + 1) * 128],
                            wv_int[:, ht, :],
                            start=(ht == 0), stop=(ht == 15),
                        )
                    gt = tt * 4 + tc4
                    vtmp = qpool.tile([128, OS], f32, name="vtmp", tag="qs_f32")
                    nc.scalar.activation(vtmp[:], ps[:], AF.Copy, scale=sxwb[:, 2:3])
                    vdq_t = qpool.tile([128, OS], f32, name="vdqt", tag="qs_f32b")
                    nc.vector.tensor_add(vdq_t[:], vtmp[:], bv_sb[:])
                    nc.vector.tensor_reduce(
                        va_parts[:, gt : gt + 1], vdq_t[:], AX.X, OP.max,
                        apply_absolute_value=True,
                    )
                    nc.sync.dma_start(out=vdq[gt], in_=vdq_t[:])
                # q, k
                for w, wint, dst, bias_sb, scol, aparts in (
                    (0, wq_int, q_deq, bq_sb, 0, qa_parts),
                    (1, wk_int, k_deq, bk_sb, 1, ka_parts),
                ):
                    for ot in range(HPC):
                        ps = qk_psum.tile([128, TLOC], f32, name="ps_qk")
                        for ht in range(16):
                            nc.tensor.matmul(
                                ps[:], wint[:, ht, ot * 128 : (ot + 1) * 128],
                                xtile(ht),
                                start=(ht == 0), stop=(ht == 15),
                            )
                        nc.scalar.activation(
                            dst[:, ot, tt * TLOC : (tt + 1) * TLOC], ps[:],
                            AF.Identity, scale=sxwb[:, scol : scol + 1],
                            bias=bias_sb[:, ot : ot + 1],
                        )
                        nc.vector.tensor_reduce(
                            qa_parts[:, tt * 2 + ot : tt * 2 + ot + 1]
                            if w == 0
                            else ka_parts[:, tt * 2 + ot : tt * 2 + ot + 1],
                            dst[:, ot, tt * TLOC : (tt + 1) * TLOC], AX.X, OP.max,
                            apply_absolute_value=True,
                        )

        for i, prt in enumerate((qa_parts, ka_parts, va_parts)):
            acol = pcols(1)
            nc.vector.tensor_reduce(acol, prt, AX.X, OP.max)
            ag = p_reduce_max(acol)
            nc.sync.dma_start(out=ar2_i[:, i : i + 1], in_=ag[0:1, :])
        nc.gpsimd.collective_compute(
            "AllReduce", OP.max, replica_groups=allg,
            ins=[ar2_i[:].opt()], outs=[ar2_o[:].opt()],
        )

        # scales from AR2: [q, k, v]
        g3 = cols(3)[0:1, :]
        nc.sync.dma_start(out=g3, in_=ar2_o[:])
        s_q = cols(1)[0:1, :]
        nc.vector.tensor_scalar(s_q, g3[:, 0:1], Q_MUL / 127.0, 1e-8, op0=OP.mult, op1=OP.max)
        qf = cols(1)[0:1, :]
        nc.vector.reciprocal(qf, s_q)
        nc.vector.tensor_scalar_mul(qf, qf, Q_MUL)
        s_kv = cols(2)[0:1, :]
        nc.vector.tensor_scalar(s_kv, g3[:, 1:3], 1.0 / 127.0, 1e-8, op0=OP.mult, op1=OP.max)
        i_kv = cols(2)[0:1, :]
        nc.vector.reciprocal(i_kv, s_kv)
        s_qk = cols(1)[0:1, :]
        nc.vector.tensor_mul(s_qk, s_q, s_kv[:, 0:1])
        qf3 = cols(3)[0:1, :]
        nc.vector.tensor_copy(qf3[:, 0:1], qf)
        nc.vector.tensor_copy(qf3[:, 1:3], i_kv)
        qf3b = bcast(qf3)
        s_qk_b = bcast(s_qk)
        neg_inv_sqk = cols(1)[0:1, :]
        nc.vector.reciprocal(neg_inv_sqk, s_qk)
        nc.vector.tensor_scalar_mul(neg_inv_sqk, neg_inv_sqk, -1.0)
        nis_b = bcast(neg_inv_sqk)

        # wp AllGather: queued after the AR2 scale broadcasts so it cannot
        # stall them on the gpsimd queue; drains during quantize/pass1.
        nc.gpsimd.collective_compute(
            "AllGather", OP.bypass, replica_groups=allg,
            ins=[wpb[:].opt()], outs=[wpg[:].opt()],
        )

        # ============ S5: quantize q, k (pair-0 chunks first) ============
        with tc.tile_pool(name="q5scratch", bufs=2) as q5pool:
            for bh in range(2):
                for ot in range(HPC):
                    tsl = slice(bh * TB, (bh + 1) * TB)
                    m = q5pool.tile([128, TB], f32, name="qm", tag="qs2_f32")
                    nc.scalar.activation(m[:], q_deq[:, ot, tsl], AF.Copy, scale=qf3b[:, 0:1])
                    rnd(q_int[:, ot, tsl], m[:])
                    m2 = q5pool.tile([128, TB], f32, name="km", tag="qs2_f32")
                    nc.scalar.activation(m2[:], k_deq[:, ot, tsl], AF.Copy, scale=qf3b[:, 1:2])
                    rnd(k_int[:, ot, tsl], m2[:])
            if debug:
                for ot in range(HPC):
                    nc.sync.dma_start(out=dbg["d_qdeq"][:, ot, :], in_=q_deq[:, ot, :])
                    nc.sync.dma_start(out=dbg["d_kdeq"][:, ot, :], in_=k_deq[:, ot, :])
        k_deq_free()
        q_deq_free()
        wv_int_free()
        wk_int_free()
        wq_int_free()
        v_int, v_int_free = tc.tile([128, 32, OS], bf16, name="v_int")

        # ============ S6: attention pass 1 (stats) ============
        # pair bp_ = (b, h): b = bp_//2, h = bp_%2
        # S (ACT accum) and M (DVE reduce) live in SEPARATE tiles so the two
        # engines never cross-serialize on a shared tile.
        stats = sbuf.tile([128, 512], f32, name="stats")
        S_tile = sbuf.tile([128, 64], f32, name="S_tile")
        M_tile = sbuf.tile([128, 64], f32, name="M_tile")
        S_all = S_tile[:, 0:64]
        M_all = M_tile[:, 0:64]
        with tc.tile_pool(name="p1_psum", bufs=2, space="PSUM") as p1_psum, \
             tc.tile_pool(name="epool", bufs=3) as e_pool:
            for bp_ in range(4):
                b_, h_ = bp_ // 2, bp_ % 2
                tb0 = b_ * TB
                for it in range(16):
                    ps = p1_psum.tile([128, TB], f32, name="ps_a")
                    for jc in range(4):
                        nc.tensor.matmul(
                            ps[:, jc * 512 : (jc + 1) * 512],
                            q_int[:, h_, tb0 + it * 128 : tb0 + (it + 1) * 128],
                            k_int[:, h_, tb0 + jc * 512 : tb0 + (jc + 1) * 512],
                            start=True, stop=True,
                        )
                    col = bp_ * 16 + it
                    E = e_pool.tile([128, TB], f32, name="E")
                    nc.scalar.activation(
                        E[:], ps[:], AF.Exp, scale=s_qk_b[:, 0:1],
                        accum_out=S_all[:, col : col + 1],
                    )
                    nc.vector.tensor_reduce(
                        M_all[:, col : col + 1], E[:], AX.X, OP.max,
                    )

        # ============ S7: AR3 + c' rows ============
        Sinv = stats[:, 128:192]
        nc.vector.reciprocal(Sinv, S_all)
        R = stats[:, 192:256]
        nc.vector.tensor_mul(R, M_all, Sinv)
        ra = pcols(1)
        nc.vector.tensor_reduce(ra, R, AX.X, OP.max)
        rag = p_reduce_max(ra)
        nc.sync.dma_start(out=ar3_i[:], in_=rag[0:1, :])
        nc.gpsimd.collective_compute(
            "AllReduce", OP.max, replica_groups=allg,
            ins=[ar3_i[:].opt()], outs=[ar3_o[:].opt()],
        )
        # v quantization fills the AR3 wait (v only needed by pass2's S@V)
        with tc.tile_pool(name="v5scratch", bufs=3) as v5pool:
            for g8 in range(4):
                vsl8 = slice(g8 * 8, (g8 + 1) * 8)
                vback = v5pool.tile([128, TB], f32, name="vback", tag="vbk")
                nc.sync.dma_start(
                    out=vback[:].rearrange("p (g o) -> p g o", o=OS),
                    in_=vdq[vsl8].rearrange("g p o -> p g o"),
                )
                m = v5pool.tile([128, TB], f32, name="vm", tag="vsc")
                nc.scalar.activation(m[:], vback[:], AF.Copy, scale=qf3b[:, 2:3])
                rnd(
                    v_int[:, vsl8, :].rearrange("p a b -> p (a b)"),
                    m[:],
                )
        if debug:
            nc.sync.dma_start(out=dbg["d_vdeq"][:], in_=vdq[:])
        # c'_i = -ln(S_i)/s_qk in f32; PE-transpose to i-ordered rows, then
        # split into bf16 hi/mid/lo rows for the pass2 ones-matmul.
        # (a direct scatter-DMA of the transpose costs ~28us on the sync queue)
        cl = stats[:, 320:384]
        nc.scalar.activation(cl, S_all, AF.Ln)
        cpr = stats[:, 384:448]
        nc.vector.tensor_scalar(cpr, cl, nis_b[:, 0:1], None, op0=OP.mult)
        with tc.tile_pool(name="rtt_psum", bufs=1, space="PSUM") as rtt_psum, \
             tc.tile_pool(name="rtt_sb", bufs=1) as rtt_pool:
            rtp = rtt_psum.tile([64, 128], f32, name="rtp")
            nc.tensor.matmul(rtp[:], cpr, ident_sb[:], start=True, stop=True)
            rtt = rtt_pool.tile([64, 128], f32, name="rtt")
            nc.vector.tensor_copy(rtt[:], rtp[:])
            nc.sync.dma_start(
                out=rt_f32.rearrange("h (it p) -> (h it) p", p=128), in_=rtt[:]
            )

        gA = cols(1)[0:1, :]
        nc.sync.dma_start(out=gA, in_=ar3_o[:])
        s_attn = cols(1)[0:1, :]
        nc.vector.tensor_scalar(s_attn, gA, 1.0 / 127.0, 1e-8, op0=OP.mult, op1=OP.max)
        lnsa = cols(1)[0:1, :]
        nc.scalar.activation(lnsa, s_attn, AF.Ln)
        nc.vector.tensor_scalar_mul(lnsa, lnsa, -1.0)
        eb_b = bcast(lnsa)
        s_av = cols(1)[0:1, :]
        nc.vector.tensor_mul(s_av, s_attn, s_kv[:, 1:2])
        s_av_b = bcast(s_av)

        if debug:
            nc.sync.dma_start(out=dbg["d_S"][:], in_=S_all)
            nc.sync.dma_start(out=dbg["d_M"][:], in_=M_all)

        # ============ S8: pass 2 + S@V ============
        out_T, out_T_free = tc.tile([128, HPC, NT], f32, name="out_T")
        oa_parts = pcols(8)
        SV_LAG = 3  # S@V matmuls trail the QK->exp->round pipeline by this many
        with tc.tile_pool(name="p2_psum", bufs=2, space="PSUM") as p2_psum, \
             tc.tile_pool(name="sv_psum", bufs=2, space="PSUM") as sv_psum, \
             tc.tile_pool(name="pint", bufs=SV_LAG + 2) as pint_pool, \
             tc.tile_pool(name="ps_scr", bufs=4) as ps_scr, \
             tc.tile_pool(name="cbpool", bufs=2) as cb_pool, \
             tc.tile_pool(name="crpool", bufs=2) as cr_pool:
            for bp_ in range(4):
                b_, h_ = bp_ // 2, bp_ % 2
                tb0 = b_ * TB
                crow = cr_pool.tile([1, TB], f32, name="crow")
                nc.sync.dma_start(out=crow[:], in_=rt_f32[bp_ : bp_ + 1, :])
                cb = cb_pool.tile([128, TB], f32, name="cb")
                nc.gpsimd.partition_broadcast(cb[:], crow[:])
                for ih in range(2):
                    isl = slice(tb0 + ih * 1024, tb0 + (ih + 1) * 1024)
                    csl = slice(ih * 1024, (ih + 1) * 1024)
                    sv = sv_psum.tile([128, 1024], f32, name="sv")
                    pending = []
                    for jt in range(16):
                        ps2 = p2_psum.tile([128, 1024], f32, name="ps2")
                        ktile = k_int[:, h_, tb0 + jt * 128 : tb0 + (jt + 1) * 128]
                        qsl = q_int[:, h_, isl]
                        for hf in range(2):
                            sl = slice(hf * 512, (hf + 1) * 512)
                            nc.tensor.matmul(
                                ps2[:, sl], ktile, qsl[:, sl],
                                start=True, stop=True,
                            )
                        PSX = ps_scr.tile([128, 1024], f32, name="PSX", tag="psx")
                        nc.vector.tensor_add(PSX[:], ps2[:], cb[:, csl])
                        PS = ps_scr.tile([128, 1024], f32, name="PS", tag="pse")
                        nc.scalar.activation(
                            PS[:], PSX[:], AF.Exp,
                            scale=s_qk_b[:, 0:1], bias=eb_b[:, 0:1],
                        )
                        pi = pint_pool.tile([128, 1024], bf16, name="pi")
                        # alternate the round between DVE and the otherwise-idle
                        # GpSimd (1-input ops run near line rate there), so DVE
                        # (the pass2 pacer: PSUM add + round) sheds half the work
                        if jt % 2 == 0:
                            rnd(pi[:], PS[:])
                        else:
                            nc.gpsimd.tensor_scalar(
                                pi[:], PS[:], RMAGIC, RMAGIC,
                                op0=OP.add, op1=OP.subtract,
                            )
                        vtile = v_int[:, b_ * 16 + jt, h_ * 128 : (h_ + 1) * 128]
                        pending.append([
                            (
                                (sv[:, hf * 512 : (hf + 1) * 512], vtile,
                                 pi[:, hf * 512 : (hf + 1) * 512]),
                                dict(start=(jt == 0), stop=(jt == 15)),
                            )
                            for hf in range(2)
                        ])
                        if len(pending) > SV_LAG:
                            for args, kw in pending.pop(0):
                                nc.tensor.matmul(*args, **kw)
                    for grp_mm in pending:
                        for args, kw in grp_mm:
                            nc.tensor.matmul(*args, **kw)
                    col = bp_ * 2 + ih
                    nc.vector.tensor_scalar(
                        out_T[:, h_, isl], sv[:], s_av_b[:, 0:1], None, op0=OP.mult
                    )
                    nc.vector.tensor_reduce(
                        oa_parts[:, col : col + 1], out_T[:, h_, isl], AX.X, OP.max,
                        apply_absolute_value=True,
                    )

        # ============ S9: out amax -> AR4 -> quantize -> A2A ============
        oc_ = pcols(1)
        nc.vector.tensor_reduce(oc_, oa_parts, AX.X, OP.max)
        ocg = p_reduce_max(oc_)
        nc.sync.dma_start(out=ar4_i[:], in_=ocg[0:1, :])
        nc.gpsimd.collective_compute(
            "AllReduce", OP.max, replica_groups=allg,
            ins=[ar4_i[:].opt()], outs=[ar4_o[:].opt()],
        )
        gO = cols(1)[0:1, :]
        nc.sync.dma_start(out=gO, in_=ar4_o[:])
        s_out = cols(1)[0:1, :]
        nc.vector.tensor_scalar(s_out, gO, 1.0 / 127.0, 1e-8, op0=OP.mult, op1=OP.max)
        i_out = cols(1)[0:1, :]
        nc.vector.reciprocal(i_out, s_out)
        io_b = bcast(i_out)
        s_op = cols(1)[0:1, :]
        nc.vector.tensor_mul(s_op, s_out, s5[:, 4:5])
        s_op_b = bcast(s_op)

        with tc.tile_pool(name="q9scratch", bufs=2) as q9pool:
            for h_ in range(HPC):
                for b_ in range(2):
                    tsl = slice(b_ * TB, (b_ + 1) * TB)
                    m = q9pool.tile([128, TB], f32, name="om", tag="qs9_f32")
                    nc.scalar.activation(m[:], out_T[:, h_, tsl], AF.Copy, scale=io_b[:, 0:1])
                    oi = q9pool.tile([128, TB], bf16, name="oi", tag="qs9_bf16")
                    rnd(oi[:], m[:])
                    nc.sync.dma_start(
                        out=a2a_i[4 * b_ : 4 * (b_ + 1), h_].rearrange("r p t -> p r t"),
                        in_=oi[:].rearrange("p (r t) -> p r t", t=TLOC),
                    )
                if debug:
                    nc.sync.dma_start(out=dbg["d_outT"][:, h_, :], in_=out_T[:, h_, :])
        nc.gpsimd.collective_compute(
            "AllToAll", OP.bypass, replica_groups=allg,
            ins=[a2a_i[:].opt()], outs=[a2a_o[:].opt()],
        )

        # ============ S10: output projection (token-parallel) ============
        out_T_free()
        out_r = out_ext.rearrange("(ot p) t -> p ot t", p=128)
        with tc.tile_pool(name="ogp", bufs=1) as og_pool, \
             tc.tile_pool(name="p7_psum", bufs=4, space="PSUM") as p7_psum, \
             tc.tile_pool(name="wcol7", bufs=3) as wcol_pool7, \
             tc.tile_pool(name="fin", bufs=3) as fin_pool:
            og = og_pool.tile([128, 16, TLOC], bf16, name="og")
            nc.sync.dma_start(out=og[:], in_=a2a_o.rearrange("s h p t -> p (s h) t"))
            for ot in range(16):
                wcol = wcol_pool7.tile([128, 16, 128], bf16, name="wcol")
                nc.sync.dma_start(out=wcol[:], in_=wpg[ot // 2, ot % 2])
                ps = p7_psum.tile([128, TLOC], f32, name="ps_p")
                for ht in range(16):
                    nc.tensor.matmul(
                        ps[:], wcol[:, ht, :], og[:, ht, :],
                        start=(ht == 0), stop=(ht == 15),
                    )
                fin = fin_pool.tile([128, TLOC], f32, name="fin")
                nc.scalar.activation(
                    fin[:], ps[:], AF.Identity,
                    scale=s_op_b[:, 0:1], bias=bp_sb[:, ot : ot + 1],
                )
                nc.sync.dma_start(out=out_r[:, ot, :], in_=fin[:])

        v_int_free()
        k_int_free()
        q_int_free()

        if debug:
            sct = cols(16)[0:1, :]
            nc.vector.tensor_copy(sct[:, 0:5], s5)
            nc.vector.tensor_copy(sct[:, 5:6], s_q)
            nc.vector.tensor_copy(sct[:, 6:8], s_kv)
            nc.vector.tensor_copy(sct[:, 8:9], s_attn)
            nc.vector.tensor_copy(sct[:, 9:10], s_out)
            nc.sync.dma_start(out=dbg["d_scales"][:], in_=sct)

    nc.compile()
    return nc


def _get_compiled(debug=False):
    if debug not in _COMPILED:
        _COMPILED[debug] = _build(debug)
    return _COMPILED[debug]


def make_in_maps(hidden_states, Wq, bq, Wk, bk, Wv, bv, Wp, bp):
    hs = np.asarray(hidden_states, dtype=np.float32)
    wT = [
        np.ascontiguousarray(np.asarray(W, np.float32).T)
        for W in (Wq, Wk, Wv, Wp)
    ]
    bp_t = np.ascontiguousarray(np.asarray(bp, np.float32).reshape(16, 128).T)
    in_maps = []
    for c in range(NC):
        b = c // GROUP
        g = c % GROUP
        osl = slice(c * OS, (c + 1) * OS)
        x_Tc = np.ascontiguousarray(hs[b, g * TLOC : (g + 1) * TLOC, :].T)
        wqkv = np.ascontiguousarray(
            np.stack([wT[w][:, osl] for w in range(3)], axis=0)
        )
        wp_slc = np.ascontiguousarray(wT[3][:, c * OSL : (c + 1) * OSL])
        bqk_c = np.ascontiguousarray(
            np.stack(
                [
                    np.asarray(bq, np.float32)[osl].reshape(HPC, 128).T,
                    np.asarray(bk, np.float32)[osl].reshape(HPC, 128).T,
                ],
                axis=0,
            )
        )
        bv_bc = np.ascontiguousarray(
            np.broadcast_to(np.asarray(bv, np.float32)[None, osl], (128, OS))
        )
        in_maps.append(
            {"x_T": x_Tc, "wqkv": wqkv, "wp_sl": wp_slc, "bqk": bqk_c,
             "bv_b": bv_bc, "bp_t": bp_t, "ident": np.eye(128, dtype=np.float32)}
        )
    return in_maps


def kernel(hidden_states, Wq, bq, Wk, bk, Wv, bv, Wp, bp):
    from concourse.bass_utils import run_bass_kernel_spmd

    debug = bool(int(os.environ.get("KERNEL_DEBUG", "0")))
    trace = bool(int(os.environ.get("KERNEL_TRACE", "0")))
    nc = _get_compiled(debug=debug)
    in_maps = make_in_maps(hidden_states, Wq, bq, Wk, bk, Wv, bv, Wp, bp)
    res = run_bass_kernel_spmd(nc, in_maps, core_ids=list(range(NC)), trace=trace)
    kernel.last_exec_time_ns = res.exec_time_ns
    kernel.last_results = res.results
    kernel.last_res = res

    out = np.empty((B, S, H), dtype=np.float32)
    for c in range(NC):
        b = c // GROUP
        g = c % GROUP
        out[b, g * TLOC : (g + 1) * TLOC, :] = res.results[c]["out"].T
    return out


kernel.last_exec_time_ns = None
kernel.last_results = None
kernel.last_res = None


# revision 57
# speedup vs baseline: 1.7717x; 1.7717x over previous
"""Distributed TRN2 Bass kernel for a dynamic-int8-quantized transformer encoder.

v2: column-parallel sharding, 2 heads x both batches per core.
  - core c: heads {2c, 2c+1} (256 qkv out dims), io token slice = flat block c
    (batch c//4, tokens (c%4)*512..+512).
  - x quantized per-core on its 512-token slice, AllGathered all-8 (RDH) as
    integer-valued bf16; Wq/Wk/Wv column slices stay LOCAL (no weight
    AllGather); Wp quantized sharded + AllGathered (overlapped mid-kernel).
  - projections column-parallel: q/k/v for ALL 4096 tokens x this core's
    2 heads; attention fully local per (batch, head) - 4 pairs per core.
  - after attention: AllToAll (all-8) redistributes quantized out to
    token-parallel; final projection token-parallel vs gathered Wp.
  - int8 fake-quant matmuls computed exactly as integer-valued bf16 matmuls
    (fp32 PSUM), dequant scales on PSUM eviction; global per-tensor amaxes via
    tiny AllReduce-max collectives.
  - softmax without max-subtraction; pass2 recomputes QK^T transposed with an
    extra k=3 ones-matmul adding c'_i = -ln(S_i)/s_qk (bf16 hi+mid+lo split),
    so round(exp(s_qk*(a_int + c'_i) - ln s_attn)) yields int8 probs directly.
  - round-to-nearest-even: x + 1.5*2^23 - 1.5*2^23 in fp32 (matches jnp.round).
"""

import math
import os
import sys
from contextlib import ExitStack

import numpy as np

sys.path.insert(0, "/opt/trn_rl_repo")

B = 2
S = 2048
H = 2048
NH = 16
D = 128
NC = 8
GROUP = 4          # io-slice cores per batch
HPC = 2            # heads per core
OS = 256           # per-core qkv out-dim slice (2 heads)
NT = 4096          # total tokens (B*S)
TB = 2048          # tokens per batch
TLOC = 512         # io token slice per core
OSL = 256          # per-core Wp out-dim slice
Q_MUL = 1.0 / math.sqrt(D)
RMAGIC = 12582912.0  # 1.5 * 2**23

_COMPILED = {}


def _build(debug=False):
    import concourse.mybir as mybir
    import concourse.tile as tile
    from concourse import bacc
    from concourse import bass_isa

    f32 = mybir.dt.float32
    bf16 = mybir.dt.bfloat16
    AF = mybir.ActivationFunctionType
    OP = mybir.AluOpType
    AX = mybir.AxisListType

    nc = bacc.Bacc(None, target_bir_lowering=False, debug=False, num_devices=NC)

    x_T = nc.declare_dram_parameter("x_T", [H, TLOC], f32, isOutput=False)
    wqkv = nc.declare_dram_parameter("wqkv", [3, H, OS], f32, isOutput=False)
    wp_sl = nc.declare_dram_parameter("wp_sl", [H, OSL], f32, isOutput=False)
    bqk = nc.declare_dram_parameter("bqk", [2, 128, HPC], f32, isOutput=False)
    bv_b = nc.declare_dram_parameter("bv_b", [128, OS], f32, isOutput=False)
    bp_t = nc.declare_dram_parameter("bp_t", [128, 16], f32, isOutput=False)
    ident = nc.declare_dram_parameter("ident", [128, 128], f32, isOutput=False)
    out_ext = nc.declare_dram_parameter("out", [H, TLOC], f32, isOutput=True)
    dbg = {}
    if debug:
        for name, shape in [
            ("d_scales", [1, 16]),
            ("d_S", [128, 64]), ("d_M", [128, 64]),
            ("d_qdeq", [128, HPC, NT]), ("d_kdeq", [128, HPC, NT]),
            ("d_vdeq", [32, 128, OS]), ("d_outT", [128, HPC, NT]),
        ]:
            dbg[name] = nc.declare_dram_parameter(name, shape, f32, isOutput=True)
        for name, shape in [
            ("d_xt", [128, 16, TLOC]), ("d_wq", [128, 16, OS]),
        ]:
            dbg[name] = nc.declare_dram_parameter(name, shape, bf16, isOutput=True)

    allg = [list(range(NC))]

    with tile.TileContext(nc) as tc, ExitStack() as top:
        dram = top.enter_context(tc.tile_pool(name="dram", bufs=1, space="DRAM"))
        # x AllGather bounce (int8 transport: half the wire bytes; SWDGE
        # casts back to bf16 on readback), all-8 gather
        i8 = mybir.dt.int8
        xq_b = dram.tile([16, 128, TLOC], i8, name="xqb")
        xg = dram.tile([NC, 16, 128, TLOC], i8, addr_space="Shared", name="xg")
        # v_deq spill (f32, streamed back for quantization after AR2)
        vdq = dram.tile([32, 128, OS], f32, name="vdq")
        # wp bounce + gather (stationary-tiled 4KB read rows)
        wpb = dram.tile([2, 128, 16, 128], bf16, name="wpb")
        wpg = dram.tile([NC, 2, 128, 16, 128], bf16, addr_space="Shared", name="wpg")
        # A2A for out_int redistribution (all-8)
        a2a_i = dram.tile([NC, HPC, 128, TLOC], bf16, name="a2ai")
        a2a_o = dram.tile([NC, HPC, 128, TLOC], bf16, name="a2ao")
        # c' rows (f32) for pass2 column bias: [bh-pair][i]
        rt_f32 = dram.tile([4, TB], f32, name="rtf32")
        # AllReduce bounces
        ar1a_i = dram.tile([1, 3], f32)
        ar1a_o = dram.tile([1, 3], f32, addr_space="Shared")
        ar1b_i = dram.tile([1, 2], f32)
        ar1b_o = dram.tile([1, 2], f32, addr_space="Shared")
        ar2_i = dram.tile([1, 3], f32)
        ar2_o = dram.tile([1, 3], f32, addr_space="Shared")
        ar3_i = dram.tile([1, 1], f32)
        ar3_o = dram.tile([1, 1], f32, addr_space="Shared")
        ar4_i = dram.tile([1, 1], f32)
        ar4_o = dram.tile([1, 1], f32, addr_space="Shared")

        const = top.enter_context(tc.tile_pool(name="const", bufs=1))
        sc = top.enter_context(tc.tile_pool(name="scal", bufs=1))
        sbuf = top.enter_context(tc.tile_pool(name="sbuf_main", bufs=1))
        qpool = top.enter_context(tc.tile_pool(name="qscratch", bufs=3))

        ones3 = const.tile([3, 128], bf16)
        nc.vector.memset(ones3[:], 1.0)

        scal = sc.tile([128, 96], f32, name="scal")
        _col = [0]

        def cols(n):
            c0 = _col[0]
            _col[0] += n
            assert _col[0] <= 96
            return scal[:, c0 : c0 + n]

        parts = sc.tile([128, 192], f32, name="parts")
        _pcol = [0]

        def pcols(n):
            c0 = _pcol[0]
            _pcol[0] += n
            assert _pcol[0] <= 192
            return parts[:, c0 : c0 + n]

        def p_reduce_max(part_col):
            red = cols(1)
            nc.gpsimd.partition_all_reduce(
                red, part_col, channels=128, reduce_op=bass_isa.ReduceOp.max
            )
            return red

        def bcast(src1n):
            b = cols(src1n.shape[-1])
            nc.gpsimd.partition_broadcast(b, src1n)
            return b

        def rnd(out_ap, in_ap):
            nc.vector.tensor_scalar(
                out_ap, in_ap, RMAGIC, RMAGIC, op0=OP.add, op1=OP.subtract
            )

        # persistent tiles (stack order: freed in reverse)
        q_int, q_int_free = tc.tile([128, HPC, NT], bf16, name="q_int")
        k_int, k_int_free = tc.tile([128, HPC, NT], bf16, name="k_int")
        wq_int, wq_int_free = tc.tile([128, 16, OS], bf16, name="wq_int")
        wk_int, wk_int_free = tc.tile([128, 16, OS], bf16, name="wk_int")
        wv_int, wv_int_free = tc.tile([128, 16, OS], bf16, name="wv_int")

        # ============ S0: load x + W slices resident; amaxes; AR1 ============
        w_f32, w_f32_free = tc.tile([128, 3, 16 * OS], f32, name="w_f32")
        wp_f32, wp_f32_free = tc.tile([128, 16 * OSL], f32, name="wp_f32")
        x_f32, x_f32_free = tc.tile([128, 16, TLOC], f32, name="x_f32")

        nc.sync.dma_start(out=x_f32[:], in_=x_T.rearrange("(ht p) t -> p ht t", p=128))
        nc.sync.dma_start(
            out=w_f32[:].rearrange("p w (ht o) -> p w ht o", o=OS),
            in_=wqkv.rearrange("w (ht p) o -> p w ht o", p=128),
        )
        nc.sync.dma_start(
            out=wp_f32[:].rearrange("p (ht o) -> p ht o", o=OSL),
            in_=wp_sl.rearrange("(ht p) o -> p ht o", p=128),
        )

        xa = pcols(1)
        nc.vector.tensor_reduce(
            xa, x_f32[:].rearrange("p a b -> p (a b)"), AX.X, OP.max,
            apply_absolute_value=True,
        )
        xag = p_reduce_max(xa)
        nc.sync.dma_start(out=ar1a_i[:, 0:1], in_=xag[0:1, :])
        for w, dst_ap in ((0, ar1a_i[:, 1:2]), (1, ar1a_i[:, 2:3])):
            wa = pcols(1)
            nc.vector.tensor_reduce(
                wa, w_f32[:, w, :], AX.X, OP.max, apply_absolute_value=True
            )
            wag = p_reduce_max(wa)
            nc.sync.dma_start(out=dst_ap, in_=wag[0:1, :])
        nc.gpsimd.collective_compute(
            "AllReduce", OP.max, replica_groups=allg,
            ins=[ar1a_i[:].opt()], outs=[ar1a_o[:].opt()],
        )
        # wv / wp amaxes (AR1b emitted AFTER the x AllGather on the cc queue)
        wa = pcols(1)
        nc.vector.tensor_reduce(
            wa, w_f32[:, 2, :], AX.X, OP.max, apply_absolute_value=True
        )
        wag = p_reduce_max(wa)
        nc.sync.dma_start(out=ar1b_i[:, 0:1], in_=wag[0:1, :])
        wa2 = pcols(1)
        nc.vector.tensor_reduce(
            wa2, wp_f32[:], AX.X, OP.max, apply_absolute_value=True
        )
        wag2 = p_reduce_max(wa2)
        nc.sync.dma_start(out=ar1b_i[:, 1:2], in_=wag2[0:1, :])

        # scales: slots [x, wq, wk, wv, wp] — x/wq/wk part right after AR1a
        g5 = cols(5)[0:1, :]
        s5 = cols(5)[0:1, :]
        i5 = cols(5)[0:1, :]
        i5b = cols(5)
        sxw = cols(3)[0:1, :]   # s_x*s_wq, s_x*s_wk, s_x*s_wv
        sxwb = cols(3)

        nc.sync.dma_start(out=g5[:, 0:3], in_=ar1a_o[:])
        nc.vector.tensor_scalar(
            s5[:, 0:3], g5[:, 0:3], 1.0 / 127.0, 1e-8, op0=OP.mult, op1=OP.max
        )
        nc.vector.reciprocal(i5[:, 0:3], s5[:, 0:3])
        nc.gpsimd.partition_broadcast(i5b[:, 0:3], i5[:, 0:3])
        nc.vector.tensor_mul(sxw[:, 0:1], s5[:, 0:1], s5[:, 1:2])
        nc.vector.tensor_mul(sxw[:, 1:2], s5[:, 0:1], s5[:, 2:3])
        nc.gpsimd.partition_broadcast(sxwb[:, 0:2], sxw[:, 0:2])

        # ============ S1: quantize x slice -> bounce -> AGx ============
        for ch in range(4):
            xsl = x_f32[:, ch * 4 : (ch + 1) * 4, :].rearrange("p a b -> p (a b)")
            xm = qpool.tile([128, TB], f32, name="xm", tag="qs_f32")
            nc.scalar.activation(xm[:], xsl, AF.Copy, scale=i5b[:, 0:1])
            xi = qpool.tile([128, TB], i8, name="xi", tag="qs_i8")
            rnd(xi[:], xm[:])
            for hh in range(4):
                nc.sync.dma_start(
                    out=xq_b[ch * 4 + hh],
                    in_=xi[:, hh * TLOC : (hh + 1) * TLOC],
                )
        nc.gpsimd.collective_compute(
            "AllGather", OP.bypass, replica_groups=allg,
            ins=[xq_b[:].opt()], outs=[xg[:].opt()],
        )
        x_f32_free()

        # AR1b on the cc queue after AGx's trigger; wire time overlaps AGx/quant
        nc.gpsimd.collective_compute(
            "AllReduce", OP.max, replica_groups=allg,
            ins=[ar1b_i[:].opt()], outs=[ar1b_o[:].opt()],
        )
        nc.sync.dma_start(out=g5[:, 3:5], in_=ar1b_o[:])
        nc.vector.tensor_scalar(
            s5[:, 3:5], g5[:, 3:5], 1.0 / 127.0, 1e-8, op0=OP.mult, op1=OP.max
        )
        nc.vector.reciprocal(i5[:, 3:5], s5[:, 3:5])
        nc.gpsimd.partition_broadcast(i5b[:, 3:5], i5[:, 3:5])
        nc.vector.tensor_mul(sxw[:, 2:3], s5[:, 0:1], s5[:, 3:4])
        nc.gpsimd.partition_broadcast(sxwb[:, 2:3], sxw[:, 2:3])

        # ============ S2: quantize weights (big chunks, from SBUF) ============
        for w, dst, scol in ((0, wq_int, 1), (1, wk_int, 2)):
            for ch in range(2):
                csl = slice(ch * 8 * OS, (ch + 1) * 8 * OS)
                wm = qpool.tile([128, TB], f32, name="wm", tag="qs_f32")
                nc.scalar.activation(
                    wm[:], w_f32[:, w, csl], AF.Copy, scale=i5b[:, scol : scol + 1]
                )
                rnd(
                    dst[:, ch * 8 : (ch + 1) * 8, :].rearrange("p a b -> p (a b)"),
                    wm[:],
                )
        for ch in range(2):
            csl = slice(ch * 8 * OS, (ch + 1) * 8 * OS)
            wm = qpool.tile([128, TB], f32, name="wmv", tag="qs_f32")
            nc.scalar.activation(wm[:], w_f32[:, 2, csl], AF.Copy, scale=i5b[:, 3:4])
            rnd(
                wv_int[:, ch * 8 : (ch + 1) * 8, :].rearrange("p a b -> p (a b)"),
                wm[:],
            )
        # wp slice -> DRAM bounce (AllGather queued after AR2)
        wpb_r = wpb.rearrange("half p ht o -> p half ht o")
        for ch in range(2):
            csl = slice(ch * 8 * OSL, (ch + 1) * 8 * OSL)
            wm = qpool.tile([128, TB], f32, name="wmp", tag="qs_f32")
            nc.scalar.activation(wm[:], wp_f32[:, csl], AF.Copy, scale=i5b[:, 4:5])
            wi = qpool.tile([128, TB], bf16, name="wip", tag="qs_bf16")
            rnd(wi[:], wm[:])
            for hh in range(8):
                nc.sync.dma_start(
                    out=wpb_r[:, :, ch * 8 + hh, :],
                    in_=wi[:, hh * OSL : (hh + 1) * OSL].rearrange(
                        "p (half o) -> p half o", o=128
                    ),
                )
        wp_f32_free()
        w_f32_free()

        # ============ S4: projections (v, q, k) column-parallel ============
        q_deq, q_deq_free = tc.tile([128, HPC, NT], f32, name="q_deq")
        k_deq, k_deq_free = tc.tile([128, HPC, NT], f32, name="k_deq")
        bq_sb = const.tile([128, HPC], f32)
        nc.sync.dma_start(out=bq_sb[:], in_=bqk[0])
        bk_sb = const.tile([128, HPC], f32)
        nc.sync.dma_start(out=bk_sb[:], in_=bqk[1])
        bv_sb = const.tile([128, OS], f32)
        nc.sync.dma_start(out=bv_sb[:], in_=bv_b[:, :])
        bp_sb = const.tile([128, 16], f32)
        nc.sync.dma_start(out=bp_sb[:], in_=bp_t[:, :])
        ident_sb = const.tile([128, 128], f32)
        nc.sync.dma_start(out=ident_sb[:], in_=ident[:, :])

        qa_parts = pcols(16)
        ka_parts = pcols(16)
        va_parts = pcols(32)

        with tc.tile_pool(name="xt", bufs=2) as xt_pool, \
             tc.tile_pool(name="v_psum", bufs=2, space="PSUM") as v_psum, \
             tc.tile_pool(name="qk_psum", bufs=6, space="PSUM") as qk_psum:
            for tt in range(NC):
                xt = xt_pool.tile([128, 16, TLOC], bf16, name="xt")
                nc.gpsimd.dma_start(
                    out=xt[:], in_=xg[tt].rearrange("h p t -> p h t")
                )
                if debug and tt == 0:
                    nc.sync.dma_start(out=dbg["d_xt"][:], in_=xt[:])
                    nc.sync.dma_start(out=dbg["d_wq"][:], in_=wq_int[:])

                def xtile(ht):
                    return xt[:, ht, :]

                # v first: out [tok, o] per 128-token block
                for tc4 in range(4):
                    ps = v_psum.tile([128, OS], f32, name="ps_v")
                    for ht in range(16):
                        nc.tensor.matmul(
                            ps[:], xtile(ht)[:, tc4 * 128 : (tc4 + 1) * 128],
                            wv_int[:, ht, :],
                            start=(ht == 0), stop=(ht == 15),
                        )
                    gt = tt * 4 + tc4
                    vtmp = qpool.tile([128, OS], f32, name="vtmp", tag="qs_f32")
                    nc.scalar.activation(vtmp[:], ps[:], AF.Copy, scale=sxwb[:, 2:3])
                    vdq_t = qpool.tile([128, OS], f32, name="vdqt", tag="qs_f32b")
                    nc.vector.tensor_add(vdq_t[:], vtmp[:], bv_sb[:])
                    nc.vector.tensor_reduce(
                        va_parts[:, gt : gt + 1], vdq_t[:], AX.X, OP.max,
                        apply_absolute_value=True,
                    )
                    nc.sync.dma_start(out=vdq[gt], in_=vdq_t[:])
                # q, k
                for w, wint, dst, bias_sb, scol, aparts in (
                    (0, wq_int, q_deq, bq_sb, 0, qa_parts),
                    (1, wk_int, k_deq, bk_sb, 1, ka_parts),
                ):
                    for ot in range(HPC):
                        ps = qk_psum.tile([128, TLOC], f32, name="ps_qk")
                        for ht in range(16):
                            nc.tensor.matmul(
                                ps[:], wint[:, ht, ot * 128 : (ot + 1) * 128],
                                xtile(ht),
                                start=(ht == 0), stop=(ht == 15),
                            )
                        nc.scalar.activation(
                            dst[:, ot, tt * TLOC : (tt + 1) * TLOC], ps[:],
                            AF.Identity, scale=sxwb[:, scol : scol + 1],
                            bias=bias_sb[:, ot : ot + 1],
                        )
                        nc.vector.tensor_reduce(
                            qa_parts[:, tt * 2 + ot : tt * 2 + ot + 1]
                            if w == 0
                            else ka_parts[:, tt * 2 + ot : tt * 2 + ot + 1],
                            dst[:, ot, tt * TLOC : (tt + 1) * TLOC], AX.X, OP.max,
                            apply_absolute_value=True,
                        )

        for i, prt in enumerate((qa_parts, ka_parts, va_parts)):
            acol = pcols(1)
            nc.vector.tensor_reduce(acol, prt, AX.X, OP.max)
            ag = p_reduce_max(acol)
            nc.sync.dma_start(out=ar2_i[:, i : i + 1], in_=ag[0:1, :])
        nc.gpsimd.collective_compute(
            "AllReduce", OP.max, replica_groups=allg,
            ins=[ar2_i[:].opt()], outs=[ar2_o[:].opt()],
        )

        # scales from AR2: [q, k, v]
        g3 = cols(3)[0:1, :]
        nc.sync.dma_start(out=g3, in_=ar2_o[:])
        s_q = cols(1)[0:1, :]
        nc.vector.tensor_scalar(s_q, g3[:, 0:1], Q_MUL / 127.0, 1e-8, op0=OP.mult, op1=OP.max)
        qf = cols(1)[0:1, :]
        nc.vector.reciprocal(qf, s_q)
        nc.vector.tensor_scalar_mul(qf, qf, Q_MUL)
        s_kv = cols(2)[0:1, :]
        nc.vector.tensor_scalar(s_kv, g3[:, 1:3], 1.0 / 127.0, 1e-8, op0=OP.mult, op1=OP.max)
        i_kv = cols(2)[0:1, :]
        nc.vector.reciprocal(i_kv, s_kv)
        s_qk = cols(1)[0:1, :]
        nc.vector.tensor_mul(s_qk, s_q, s_kv[:, 0:1])
        qf3 = cols(3)[0:1, :]
        nc.vector.tensor_copy(qf3[:, 0:1], qf)
        nc.vector.tensor_copy(qf3[:, 1:3], i_kv)
        qf3b = bcast(qf3)
        s_qk_b = bcast(s_qk)
        neg_inv_sqk = cols(1)[0:1, :]
        nc.vector.reciprocal(neg_inv_sqk, s_qk)
        nc.vector.tensor_scalar_mul(neg_inv_sqk, neg_inv_sqk, -1.0)
        nis_b = bcast(neg_inv_sqk)

        # wp AllGather: queued after the AR2 scale broadcasts so it cannot
        # stall them on the gpsimd queue; drains during quantize/pass1.
        nc.gpsimd.collective_compute(
            "AllGather", OP.bypass, replica_groups=allg,
            ins=[wpb[:].opt()], outs=[wpg[:].opt()],
        )

        # ============ S5: quantize q, k (pair-0 chunks first) ============
        with tc.tile_pool(name="q5scratch", bufs=2) as q5pool:
            for bh in range(2):
                for ot in range(HPC):
                    tsl = slice(bh * TB, (bh + 1) * TB)
                    m = q5pool.tile([128, TB], f32, name="qm", tag="qs2_f32")
                    nc.scalar.activation(m[:], q_deq[:, ot, tsl], AF.Copy, scale=qf3b[:, 0:1])
                    rnd(q_int[:, ot, tsl], m[:])
                    m2 = q5pool.tile([128, TB], f32, name="km", tag="qs2_f32")
                    nc.scalar.activation(m2[:], k_deq[:, ot, tsl], AF.Copy, scale=qf3b[:, 1:2])
                    rnd(k_int[:, ot, tsl], m2[:])
            if debug:
                for ot in range(HPC):
                    nc.sync.dma_start(out=dbg["d_qdeq"][:, ot, :], in_=q_deq[:, ot, :])
                    nc.sync.dma_start(out=dbg["d_kdeq"][:, ot, :], in_=k_deq[:, ot, :])
        k_deq_free()
        q_deq_free()
        wv_int_free()
        wk_int_free()
        wq_int_free()
        v_int, v_int_free = tc.tile([128, 32, OS], bf16, name="v_int")

        # ============ S6: attention pass 1 (stats) ============
        # pair bp_ = (b, h): b = bp_//2, h = bp_%2
        # S (ACT accum) and M (DVE reduce) live in SEPARATE tiles so the two
        # engines never cross-serialize on a shared tile.
        stats = sbuf.tile([128, 512], f32, name="stats")
        S_tile = sbuf.tile([128, 64], f32, name="S_tile")
        M_tile = sbuf.tile([128, 64], f32, name="M_tile")
        S_all = S_tile[:, 0:64]
        M_all = M_tile[:, 0:64]
        with tc.tile_pool(name="p1_psum", bufs=2, space="PSUM") as p1_psum, \
             tc.tile_pool(name="epool", bufs=3) as e_pool:
            for bp_ in range(4):
                b_, h_ = bp_ // 2, bp_ % 2
                tb0 = b_ * TB
                for it in range(16):
                    ps = p1_psum.tile([128, TB], f32, name="ps_a")
                    for jc in range(4):
                        nc.tensor.matmul(
                            ps[:, jc * 512 : (jc + 1) * 512],
                            q_int[:, h_, tb0 + it * 128 : tb0 + (it + 1) * 128],
                            k_int[:, h_, tb0 + jc * 512 : tb0 + (jc + 1) * 512],
                            start=True, stop=True,
                        )
                    col = bp_ * 16 + it
                    E = e_pool.tile([128, TB], f32, name="E")
                    nc.scalar.activation(
                        E[:], ps[:], AF.Exp, scale=s_qk_b[:, 0:1],
                        accum_out=S_all[:, col : col + 1],
                    )
                    nc.vector.tensor_reduce(
                        M_all[:, col : col + 1], E[:], AX.X, OP.max,
                    )

        # ============ S7: AR3 + c' rows ============
        Sinv = stats[:, 128:192]
        nc.vector.reciprocal(Sinv, S_all)
        R = stats[:, 192:256]
        nc.vector.tensor_mul(R, M_all, Sinv)
        ra = pcols(1)
        nc.vector.tensor_reduce(ra, R, AX.X, OP.max)
        rag = p_reduce_max(ra)
        nc.sync.dma_start(out=ar3_i[:], in_=rag[0:1, :])
        nc.gpsimd.collective_compute(
            "AllReduce", OP.max, replica_groups=allg,
            ins=[ar3_i[:].opt()], outs=[ar3_o[:].opt()],
        )
        # v quantization fills the AR3 wait (v only needed by pass2's S@V)
        with tc.tile_pool(name="v5scratch", bufs=3) as v5pool:
            for g8 in range(4):
                vsl8 = slice(g8 * 8, (g8 + 1) * 8)
                vback = v5pool.tile([128, TB], f32, name="vback", tag="vbk")
                nc.sync.dma_start(
                    out=vback[:].rearrange("p (g o) -> p g o", o=OS),
                    in_=vdq[vsl8].rearrange("g p o -> p g o"),
                )
                m = v5pool.tile([128, TB], f32, name="vm", tag="vsc")
                nc.scalar.activation(m[:], vback[:], AF.Copy, scale=qf3b[:, 2:3])
                rnd(
                    v_int[:, vsl8, :].rearrange("p a b -> p (a b)"),
                    m[:],
                )
        if debug:
            nc.sync.dma_start(out=dbg["d_vdeq"][:], in_=vdq[:])
        # c'_i = -ln(S_i)/s_qk in f32; PE-transpose to i-ordered rows, then
        # split into bf16 hi/mid/lo rows for the pass2 ones-matmul.
        # (a direct scatter-DMA of the transpose costs ~28us on the sync queue)
        cl = stats[:, 320:384]
        nc.scalar.activation(cl, S_all, AF.Ln)
        cpr = stats[:, 384:448]
        nc.vector.tensor_scalar(cpr, cl, nis_b[:, 0:1], None, op0=OP.mult)
        with tc.tile_pool(name="rtt_psum", bufs=1, space="PSUM") as rtt_psum, \
             tc.tile_pool(name="rtt_sb", bufs=1) as rtt_pool:
            rtp = rtt_psum.tile([64, 128], f32, name="rtp")
            nc.tensor.matmul(rtp[:], cpr, ident_sb[:], start=True, stop=True)
            rtt = rtt_pool.tile([64, 128], f32, name="rtt")
            nc.vector.tensor_copy(rtt[:], rtp[:])
            nc.sync.dma_start(
                out=rt_f32.rearrange("h (it p) -> (h it) p", p=128), in_=rtt[:]
            )

        gA = cols(1)[0:1, :]
        nc.sync.dma_start(out=gA, in_=ar3_o[:])
        s_attn = cols(1)[0:1, :]
        nc.vector.tensor_scalar(s_attn, gA, 1.0 / 127.0, 1e-8, op0=OP.mult, op1=OP.max)
        lnsa = cols(1)[0:1, :]
        nc.scalar.activation(lnsa, s_attn, AF.Ln)
        nc.vector.tensor_scalar_mul(lnsa, lnsa, -1.0)
        eb_b = bcast(lnsa)
        s_av = cols(1)[0:1, :]
        nc.vector.tensor_mul(s_av, s_attn, s_kv[:, 1:2])
        s_av_b = bcast(s_av)

        if debug:
            nc.sync.dma_start(out=dbg["d_S"][:], in_=S_all)
            nc.sync.dma_start(out=dbg["d_M"][:], in_=M_all)

        # ============ S8: pass 2 + S@V ============
        out_T, out_T_free = tc.tile([128, HPC, NT], f32, name="out_T")
        oa_parts = pcols(8)
        SV_LAG = 3  # S@V matmuls trail the QK->exp->round pipeline by this many
        with tc.tile_pool(name="p2_psum", bufs=2, space="PSUM") as p2_psum, \
             tc.tile_pool(name="sv_psum", bufs=2, space="PSUM") as sv_psum, \
             tc.tile_pool(name="pint", bufs=SV_LAG + 2) as pint_pool, \
             tc.tile_pool(name="ps_scr", bufs=4) as ps_scr, \
             tc.tile_pool(name="cbpool", bufs=2) as cb_pool, \
             tc.tile_pool(name="crpool", bufs=2) as cr_pool:
            for bp_ in range(4):
                b_, h_ = bp_ // 2, bp_ % 2
                tb0 = b_ * TB
                crow = cr_pool.tile([1, TB], f32, name="crow")
                nc.sync.dma_start(out=crow[:], in_=rt_f32[bp_ : bp_ + 1, :])
                cb = cb_pool.tile([128, TB], f32, name="cb")
                nc.gpsimd.partition_broadcast(cb[:], crow[:])
                for ih in range(2):
                    isl = slice(tb0 + ih * 1024, tb0 + (ih + 1) * 1024)
                    csl = slice(ih * 1024, (ih + 1) * 1024)
                    sv = sv_psum.tile([128, 1024], f32, name="sv")
                    pending = []
                    for jt in range(16):
                        ps2 = p2_psum.tile([128, 1024], f32, name="ps2")
                        ktile = k_int[:, h_, tb0 + jt * 128 : tb0 + (jt + 1) * 128]
                        qsl = q_int[:, h_, isl]
                        for hf in range(2):
                            sl = slice(hf * 512, (hf + 1) * 512)
                            nc.tensor.matmul(
                                ps2[:, sl], ktile, qsl[:, sl],
                                start=True, stop=True,
                            )
                        PSX = ps_scr.tile([128, 1024], f32, name="PSX", tag="psx")
                        nc.vector.tensor_add(PSX[:], ps2[:], cb[:, csl])
                        PS = ps_scr.tile([128, 1024], f32, name="PS", tag="pse")
                        nc.scalar.activation(
                            PS[:], PSX[:], AF.Exp,
                            scale=s_qk_b[:, 0:1], bias=eb_b[:, 0:1],
                        )
                        pi = pint_pool.tile([128, 1024], bf16, name="pi")
                        rnd(pi[:], PS[:])
                        vtile = v_int[:, b_ * 16 + jt, h_ * 128 : (h_ + 1) * 128]
                        pending.append([
                            (
                                (sv[:, hf * 512 : (hf + 1) * 512], vtile,
                                 pi[:, hf * 512 : (hf + 1) * 512]),
                                dict(start=(jt == 0), stop=(jt == 15)),
                            )
                            for hf in range(2)
                        ])
                        if len(pending) > SV_LAG:
                            for args, kw in pending.pop(0):
                                nc.tensor.matmul(*args, **kw)
                    for grp_mm in pending:
                        for args, kw in grp_mm:
                            nc.tensor.matmul(*args, **kw)
                    col = bp_ * 2 + ih
                    nc.vector.tensor_scalar(
                        out_T[:, h_, isl], sv[:], s_av_b[:, 0:1], None, op0=OP.mult
                    )
                    nc.vector.tensor_reduce(
                        oa_parts[:, col : col + 1], out_T[:, h_, isl], AX.X, OP.max,
                        apply_absolute_value=True,
                    )

        # ============ S9: out amax -> AR4 -> quantize -> A2A ============
        oc_ = pcols(1)
        nc.vector.tensor_reduce(oc_, oa_parts, AX.X, OP.max)
        ocg = p_reduce_max(oc_)
        nc.sync.dma_start(out=ar4_i[:], in_=ocg[0:1, :])
        nc.gpsimd.collective_compute(
            "AllReduce", OP.max, replica_groups=allg,
            ins=[ar4_i[:].opt()], outs=[ar4_o[:].opt()],
        )
        gO = cols(1)[0:1, :]
        nc.sync.dma_start(out=gO, in_=ar4_o[:])
        s_out = cols(1)[0:1, :]
        nc.vector.tensor_scalar(s_out, gO, 1.0 / 127.0, 1e-8, op0=OP.mult, op1=OP.max)
        i_out = cols(1)[0:1, :]
        nc.vector.reciprocal(i_out, s_out)
        io_b = bcast(i_out)
        s_op = cols(1)[0:1, :]
        nc.vector.tensor_mul(s_op, s_out, s5[:, 4:5])
        s_op_b = bcast(s_op)

        with tc.tile_pool(name="q9scratch", bufs=2) as q9pool:
            for h_ in range(HPC):
                for b_ in range(2):
                    tsl = slice(b_ * TB, (b_ + 1) * TB)
                    m = q9pool.tile([128, TB], f32, name="om", tag="qs9_f32")
                    nc.scalar.activation(m[:], out_T[:, h_, tsl], AF.Copy, scale=io_b[:, 0:1])
                    oi = q9pool.tile([128, TB], bf16, name="oi", tag="qs9_bf16")
                    rnd(oi[:], m[:])
                    nc.sync.dma_start(
                        out=a2a_i[4 * b_ : 4 * (b_ + 1), h_].rearrange("r p t -> p r t"),
                        in_=oi[:].rearrange("p (r t) -> p r t", t=TLOC),
                    )
                if debug:
                    nc.sync.dma_start(out=dbg["d_outT"][:, h_, :], in_=out_T[:, h_, :])
        nc.gpsimd.collective_compute(
            "AllToAll", OP.bypass, replica_groups=allg,
            ins=[a2a_i[:].opt()], outs=[a2a_o[:].opt()],
        )

        # ============ S10: output projection (token-parallel) ============
        out_T_free()
        out_r = out_ext.rearrange("(ot p) t -> p ot t", p=128)
        with tc.tile_pool(name="ogp", bufs=1) as og_pool, \
             tc.tile_pool(name="p7_psum", bufs=4, space="PSUM") as p7_psum, \
             tc.tile_pool(name="wcol7", bufs=3) as wcol_pool7, \
             tc.tile_pool(name="fin", bufs=3) as fin_pool:
            og = og_pool.tile([128, 16, TLOC], bf16, name="og")
            nc.sync.dma_start(out=og[:], in_=a2a_o.rearrange("s h p t -> p (s h) t"))
            for ot in range(16):
                wcol = wcol_pool7.tile([128, 16, 128], bf16, name="wcol")
                nc.sync.dma_start(out=wcol[:], in_=wpg[ot // 2, ot % 2])
                ps = p7_psum.tile([128, TLOC], f32, name="ps_p")
                for ht in range(16):
                    nc.tensor.matmul(
                        ps[:], wcol[:, ht, :], og[:, ht, :],
                        start=(ht == 0), stop=(ht == 15),
                    )
                fin = fin_pool.tile([128, TLOC], f32, name="fin")
                nc.scalar.activation(
                    fin[:], ps[:], AF.Identity,
                    scale=s_op_b[:, 0:1], bias=bp_sb[:, ot : ot + 1],
                )
                nc.sync.dma_start(out=out_r[:, ot, :], in_=fin[:])

        v_int_free()
        k_int_free()
        q_int_free()

        if debug:
            sct = cols(16)[0:1, :]
            nc.vector.tensor_copy(sct[:, 0:5], s5)
            nc.vector.tensor_copy(sct[:, 5:6], s_q)
            nc.vector.tensor_copy(sct[:, 6:8], s_kv)
            nc.vector.tensor_copy(sct[:, 8:9], s_attn)
            nc.vector.tensor_copy(sct[:, 9:10], s_out)
            nc.sync.dma_start(out=dbg["d_scales"][:], in_=sct)

    nc.compile()
    return nc


def _get_compiled(debug=False):
    if debug not in _COMPILED:
        _COMPILED[debug] = _build(debug)
    return _COMPILED[debug]


def make_in_maps(hidden_states, Wq, bq, Wk, bk, Wv, bv, Wp, bp):
    hs = np.asarray(hidden_states, dtype=np.float32)
    wT = [
        np.ascontiguousarray(np.asarray(W, np.float32).T)
        for W in (Wq, Wk, Wv, Wp)
    ]
    bp_t = np.ascontiguousarray(np.asarray(bp, np.float32).reshape(16, 128).T)
    in_maps = []
    for c in range(NC):
        b = c // GROUP
        g = c % GROUP
        osl = slice(c * OS, (c + 1) * OS)
        x_Tc = np.ascontiguousarray(hs[b, g * TLOC : (g + 1) * TLOC, :].T)
        wqkv = np.ascontiguousarray(
            np.stack([wT[w][:, osl] for w in range(3)], axis=0)
        )
        wp_slc = np.ascontiguousarray(wT[3][:, c * OSL : (c + 1) * OSL])
        bqk_c = np.ascontiguousarray(
            np.stack(
                [
                    np.asarray(bq, np.float32)[osl].reshape(HPC, 128).T,
                    np.asarray(bk, np.float32)[osl].reshape(HPC, 128).T,
                ],
                axis=0,
            )
        )
        bv_bc = np.ascontiguousarray(
            np.broadcast_to(np.asarray(bv, np.float32)[None, osl], (128, OS))
        )
        in_maps.append(
            {"x_T": x_Tc, "wqkv": wqkv, "wp_sl": wp_slc, "bqk": bqk_c,
             "bv_b": bv_bc, "bp_t": bp_t, "ident": np.eye(128, dtype=np.float32)}
        )
    return in_maps


def kernel(hidden_states, Wq, bq, Wk, bk, Wv, bv, Wp, bp):
    from concourse.bass_utils import run_bass_kernel_spmd

    debug = bool(int(os.environ.get("KERNEL_DEBUG", "0")))
    trace = bool(int(os.environ.get("KERNEL_TRACE", "0")))
    nc = _get_compiled(debug=debug)
    in_maps = make_in_maps(hidden_states, Wq, bq, Wk, bk, Wv, bv, Wp, bp)
    res = run_bass_kernel_spmd(nc, in_maps, core_ids=list(range(NC)), trace=trace)
    kernel.last_exec_time_ns = res.exec_time_ns
    kernel.last_results = res.results
    kernel.last_res = res

    out = np.empty((B, S, H), dtype=np.float32)
    for c in range(NC):
        b = c // GROUP
        g = c % GROUP
        out[b, g * TLOC : (g + 1) * TLOC, :] = res.results[c]["out"].T
    return out


kernel.last_exec_time_ns = None
kernel.last_results = None
kernel.last_res = None
